# revision 12
# baseline (speedup 1.0000x reference)
"""CS-TreeLSTM (BRANCH=16, DEPTH=4, IN=HID=512) on 8 Trainium2 NeuronCores.

v2 strategy (data-parallel over subtrees):
  - Each core owns 8192 leaves + 512 level-3 nodes; host finishes levels
    2..0 in float64 (0.4% of FLOPs), and additionally precomputes exactly
    (f64) the level-3 x-part pre-activations (x3@W*x + b) and the forget
    x-term fx3 = x3@Wfx + bf, which the device consumes as PSUM pre-writes.
  - Leaf u,o gate matmuls run as fp8e4 DoubleRow (2 contraction rows per
    cycle) with a two-digit hi/lo decomposition on BOTH operands (3
    digit-product terms, lo*lo dropped).  Digits share one scale per side
    (x*16, W*64) so all terms accumulate in one PSUM group; the 1/1024
    descale rides the ACT evacuation's scale argument.  The i gate (most
    error-sensitive: its error multiplies u into C which the fcsum cascade
    amplifies ~10x) stays f32r, as do the Wfh@C forget matmuls and the
    level-3 h-part.
  - Elementwise: C-path (i,u,C) fp32; H-path (o,t,H) and f bf16; sums fp32.
  - Sibling sums are grouped free-dim tensor_reduce on DVE; f*C runs on
    the otherwise idle Pool engine, which also pre-writes the fx/l3-x
    terms into PSUM so those matmul groups accumulate on top (start=False).
  - Gate PSUM groups are 4-bank [128,4,512] tiles, two in flight
    (ping-pong) = all 8 banks.
"""

import sys

sys.path.insert(0, "/opt/trn_rl_repo")

import numpy as np

BRANCH = 16
DEPTH = 4
IN = 512
HID = 512
NC_N = 8
OFFS = [0, 1, 17, 273, 4369, 69905]
N_CHUNK = 16
LEAF_PER_CORE = 8192
L3_PER_CORE = 512

# which leaf gates use fp8 DoubleRow (others f32r)
FP8_GATES = {"i": False, "u": True, "o": True}
SX = 16.0  # x digit scale
SW = 64.0  # w digit scale

_CACHE = {}


def _build_nc():
    from concourse import bacc
    import concourse.mybir as mybir
    import concourse.tile as tile

    F32 = mybir.dt.float32
    F32R = mybir.dt.float32r
    BF16 = mybir.dt.bfloat16
    FP8 = mybir.dt.float8e4
    ACTF = mybir.ActivationFunctionType
    AX = mybir.AxisListType
    OP = mybir.AluOpType
    DR = mybir.MatmulPerfMode.DoubleRow

    any8 = any(FP8_GATES.values())
    anyr = not all(FP8_GATES.values())

    nc = bacc.Bacc()

    # ---- DRAM params ----
    x8 = nc.declare_dram_parameter("x8", [128, N_CHUNK, 4, 2, 512], FP8, isOutput=False) if any8 else None
    xr = nc.declare_dram_parameter("xr", [128, N_CHUNK, 4, 512], F32, isOutput=False) if anyr else None
    w8 = {}
    wr = {}
    for g in "iuo":
        if FP8_GATES[g]:
            w8[g] = nc.declare_dram_parameter("w8" + g, [128, 4, 2, 512], FP8, isOutput=False)
        else:
            wr[g] = nc.declare_dram_parameter("wr" + g, [128, 4, 512], F32, isOutput=False)
    wfh = nc.declare_dram_parameter("wfh", [128, 4, 512], F32, isOutput=False)
    fx3b = nc.declare_dram_parameter("fx3b", [128, 4, 512], F32, isOutput=False)
    l3p = {g: nc.declare_dram_parameter("l3p" + g, [128, 4, 512], F32, isOutput=False) for g in "iuo"}
    wh = {g: nc.declare_dram_parameter("wh" + g, [128, 4, 512], F32, isOutput=False) for g in "iuo"}
    bT = {g: nc.declare_dram_parameter("bT" + g, [128, 4], F32, isOutput=False) for g in "iuo"}
    out_t = {
        "i3T": nc.declare_dram_parameter("i3T", [128, 4, 512], BF16, isOutput=True),
        "u3T": nc.declare_dram_parameter("u3T", [128, 4, 512], BF16, isOutput=True),
        "o3T": nc.declare_dram_parameter("o3T", [128, 4, 512], BF16, isOutput=True),
        "fc3T": nc.declare_dram_parameter("fc3T", [128, 4, 512], F32, isOutput=True),
    }

    from contextlib import ExitStack

    with tile.TileContext(nc) as tc, ExitStack() as ctx:
        consts = ctx.enter_context(tc.tile_pool(name="consts", bufs=1))
        s8 = ctx.enter_context(tc.tile_pool(name="s8", bufs=2))
        sr = ctx.enter_context(tc.tile_pool(name="sr", bufs=2))
        gi = ctx.enter_context(tc.tile_pool(name="gi", bufs=2))
        gu = ctx.enter_context(tc.tile_pool(name="gu", bufs=2))
        go = ctx.enter_context(tc.tile_pool(name="go", bufs=2))
        gf = ctx.enter_context(tc.tile_pool(name="gf", bufs=1))
        gC = ctx.enter_context(tc.tile_pool(name="gC", bufs=3))
        gt = ctx.enter_context(tc.tile_pool(name="gt", bufs=1))
        gH = ctx.enter_context(tc.tile_pool(name="gH", bufs=1))
        gfc = ctx.enter_context(tc.tile_pool(name="gfc", bufs=1))
        longp = ctx.enter_context(tc.tile_pool(name="longp", bufs=1))
        psum = ctx.enter_context(tc.tile_pool(name="psum", bufs=4, space="PSUM"))

        # ---- constant loads (Pool SWDGE queue, parallel desc-gen) ----
        W8s, Wrs, bTs, L3p = {}, {}, {}, {}
        for g in "iuo":
            bTs[g] = consts.tile([128, 4], F32, tag="bT" + g, name="bT" + g)
            nc.gpsimd.dma_start(out=bTs[g][:, :], in_=bT[g][:, :])
        # ordered roughly by first use
        for g in "iuo" if not FP8_GATES["i"] else "uoi":
            if FP8_GATES[g]:
                W8s[g] = consts.tile([128, 4, 2, 512], FP8, tag="w8" + g, name="w8" + g)
                nc.gpsimd.dma_start(out=W8s[g][:, :, :, :], in_=w8[g][:, :, :, :])
            else:
                Wrs[g] = consts.tile([128, 4, 512], F32R, tag="wr" + g, name="wr" + g)
                nc.gpsimd.dma_start(out=Wrs[g][:, :, :], in_=wr[g][:, :, :].bitcast(F32R))
        Wfh = consts.tile([128, 4, 512], F32R, tag="wfh", name="wfh")
        Fx3b = consts.tile([128, 4, 512], F32, tag="fx3b", name="fx3b")
        for g in "iuo":
            L3p[g] = consts.tile([128, 4, 512], F32, tag="l3p" + g, name="l3p" + g)

        # persistent accumulators
        hsum3T = longp.tile([128, 4, 512], F32R, tag="hsum3T")
        fcsum3T = longp.tile([128, 4, 512], F32, tag="fcsum3T")

        def load_x8(c):
            t = s8.tile([128, 4, 2, 512], FP8, tag="x8", name=f"x8_{c}")
            nc.sync.dma_start(out=t[:, :, :, :], in_=x8[:, c, :, :, :])
            return t

        def load_xr(c):
            t = sr.tile([128, 4, 512], F32R, tag="xr", name=f"xr_{c}")
            nc.sync.dma_start(out=t[:, :, :], in_=xr[:, c, :, :].bitcast(F32R))
            return t

        def ps_pair():
            # two 2-bank half tiles per gate group; 4-deep ring = 8 banks,
            # so the PE can run ~2 gate-halves ahead of the ACT evacuations
            return [psum.tile([128, 2, 512], F32, tag="ps", name="ps") for _ in range(2)]

        def ps_m(pp, m):
            return pp[m // 2][:, m % 2, :]

        def mm_fp8(pp, Wt, xt):
            """3-digit-term fp8 DoubleRow product into half-tile pair.
            Wt [128,4k,2(hi,lo),512m]; xt [128,4k,2(lo,hi),512n]."""
            for m in range(4):
                ms = slice(m * 128, (m + 1) * 128)
                for kp in range(2):  # hi@hi, k-pair packed
                    nc.tensor.matmul(
                        ps_m(pp, m),
                        Wt[:, 2 * kp : 2 * kp + 2, 0, ms],
                        xt[:, 2 * kp : 2 * kp + 2, 1, :],
                        start=(kp == 0), stop=False, perf_mode=DR,
                    )
                for k in range(4):  # cross: (W_hi,W_lo) x (x_lo,x_hi)
                    nc.tensor.matmul(
                        ps_m(pp, m),
                        Wt[:, k, :, ms],
                        xt[:, k, :, :],
                        start=False, stop=(k == 3), perf_mode=DR,
                    )

        def mm_f32r(pp, Wt, xt):
            for m in range(4):
                ms = slice(m * 128, (m + 1) * 128)
                for k in range(4):
                    nc.tensor.matmul(
                        ps_m(pp, m), Wt[:, k, ms], xt[:, k, :],
                        start=(k == 0), stop=(k == 3),
                    )

        def gate_mm(g, x8t, xrt):
            pp = ps_pair()
            if FP8_GATES[g]:
                mm_fp8(pp, W8s[g], x8t)
            else:
                mm_f32r(pp, Wrs[g], xrt)
            return pp

        def gate_evac(g, pp, pool, dtype):
            sc = 1.0 / (SX * SW) if FP8_GATES[g] else 1.0
            sb = pool.tile([128, 4, 512], dtype, tag=g)
            act = ACTF.Tanh if g == "u" else ACTF.Sigmoid
            for m in range(4):  # bias varies per m-tile
                nc.scalar.activation(
                    sb[:, m, :], ps_m(pp, m), act,
                    bias=bTs[g][:, m : m + 1], scale=sc,
                )
            return sb

        def fpath_mm(C_prev):
            pp = ps_pair()
            mm_f32r(pp, Wfh, C_prev)
            return pp

        def fpath_add(c, pp):
            # DVE folds the broadcast fx3(+bias) term in, off the PE path
            f_sb = gf.tile([128, 4, 512], BF16, tag="f")
            for h in range(2):
                nc.vector.tensor_tensor(
                    out=f_sb[:, 2 * h : 2 * h + 2, :].rearrange("p t (g w) -> p t g w", w=16),
                    in0=pp[h][:, :, :].rearrange("p t (g w) -> p t g w", w=16),
                    in1=Fx3b[:, 2 * h : 2 * h + 2, 32 * c : 32 * c + 32][:, :, :, None]
                    .broadcast_to([128, 2, 32, 16]),
                    op=OP.add,
                )
            return f_sb

        def fpath_rest(c, f_sb, C_prev):
            nc.scalar.activation(f_sb[:, :, :], f_sb[:, :, :], ACTF.Sigmoid)
            fC_sb = gfc.tile([128, 4, 512], F32, tag="fC")
            nc.gpsimd.tensor_mul(fC_sb[:, :, :], f_sb[:, :, :], C_prev[:, :, :].bitcast(F32))
            nc.vector.tensor_reduce(
                fcsum3T[:, :, 32 * c : 32 * c + 32],
                fC_sb[:, :, :].rearrange("p t (g w) -> p t g w", w=16),
                axis=AX.X, op=OP.add,
            )

        # ---------------- leaf phase ----------------
        # per chunk c: PE runs f-mm(c-1), i-mm(c), u-mm(c), o-mm(c); the ACT
        # queue is ordered so no evacuation sits behind sigmoid-f / tanh-C.
        pipe = None
        prevHt = None
        Whs = {}
        for c in range(N_CHUNK):
            x8t = load_x8(c) if any8 else None
            xrt = load_xr(c) if anyr else None
            if c == 0:
                nc.gpsimd.dma_start(out=Wfh[:, :, :], in_=wfh[:, :, :].bitcast(F32R))
                nc.gpsimd.dma_start(out=Fx3b[:, :, :], in_=fx3b[:, :, :])
            if 2 <= c <= 4:
                g = "iuo"[c - 2]
                nc.gpsimd.dma_start(out=L3p[g][:, :, :], in_=l3p[g][:, :, :])
            if c >= 13:
                g = "iuo"[c - 13]
                Whs[g] = sr.tile([128, 4, 512], F32R, tag="wh" + g, bufs=1, name="wh" + g)
                nc.gpsimd.dma_start(out=Whs[g][:, :, :], in_=wh[g][:, :, :].bitcast(F32R))

            if pipe is not None:
                ppf = fpath_mm(pipe[1])
            pp_i = gate_mm("i", x8t, xrt)
            if pipe is not None:
                f_sb = fpath_add(pipe[0], ppf)
            i_sb = gate_evac("i", pp_i, gi, F32)
            pp_u = gate_mm("u", x8t, xrt)
            if pipe is not None:
                fpath_rest(pipe[0], f_sb, pipe[1])
            u_sb = gate_evac("u", pp_u, gu, F32)
            pp_o = gate_mm("o", x8t, xrt)
            C_sb = gC.tile([128, 4, 512], F32R, tag="C")
            nc.vector.tensor_mul(C_sb[:, :, :], i_sb[:, :, :], u_sb[:, :, :])
            t_sb = gt.tile([128, 4, 512], BF16, tag="t")
            nc.scalar.activation(t_sb[:, :, :], C_sb[:, :, :].bitcast(F32), ACTF.Tanh)
            o_sb = gate_evac("o", pp_o, go, BF16)

            if prevHt is not None:
                # previous chunk's H product + sibling sum (Pool + DVE)
                po, pt, pc = prevHt
                H_sb = gH.tile([128, 4, 512], BF16, tag="H")
                nc.gpsimd.tensor_mul(H_sb[:, :, :], po[:, :, :], pt[:, :, :])
                with nc.allow_low_precision("f32r rounding for l3 h matmul"):
                    nc.vector.tensor_reduce(
                        hsum3T[:, :, 32 * pc : 32 * pc + 32],
                        H_sb[:, :, :].rearrange("p t (g w) -> p t g w", w=16),
                        axis=AX.X, op=OP.add,
                    )
            prevHt = (o_sb, t_sb, c)
            pipe = (c, C_sb)

        # drain the pipeline: last f-path and last H/hsum
        ppf = fpath_mm(pipe[1])
        f_sb = fpath_add(pipe[0], ppf)
        fpath_rest(pipe[0], f_sb, pipe[1])
        po, pt, pc = prevHt
        H_sb = gH.tile([128, 4, 512], BF16, tag="H")
        nc.gpsimd.tensor_mul(H_sb[:, :, :], po[:, :, :], pt[:, :, :])
        with nc.allow_low_precision("f32r rounding for l3 h matmul"):
            nc.vector.tensor_reduce(
                hsum3T[:, :, 32 * pc : 32 * pc + 32],
                H_sb[:, :, :].rearrange("p t (g w) -> p t g w", w=16),
                axis=AX.X, op=OP.add,
            )

        # ---------------- level 3 ----------------
        nc.sync.dma_start(out=out_t["fc3T"][:, :, :], in_=fcsum3T[:, :, :])
        out_pool = {"i": gi, "u": gu, "o": go}
        for g in "iuo":
            pp = ps_pair()
            mm_f32r(pp, Whs[g], hsum3T)
            pre = gC.tile([128, 4, 512], F32, tag="C", name="l3pre" + g)
            for h in range(2):
                nc.vector.tensor_tensor(
                    out=pre[:, 2 * h : 2 * h + 2, :], in0=pp[h][:, :, :],
                    in1=L3p[g][:, 2 * h : 2 * h + 2, :], op=OP.add,
                )
            sb = out_pool[g].tile([128, 4, 512], BF16, tag=g)
            act = ACTF.Tanh if g == "u" else ACTF.Sigmoid
            nc.scalar.activation(sb[:, :, :], pre[:, :, :], act)
            nc.sync.dma_start(out=out_t[g + "3T"][:, :, :], in_=sb[:, :, :])

    nc.finalize()
    return nc


def _np_sigmoid(v):
    return 1.0 / (1.0 + np.exp(-v))


def _q8(a):
    import ml_dtypes

    return np.asarray(a, np.float32).astype(ml_dtypes.float8_e4m3)


def _host_prep(x, wi_w, wo_w, wu_w, wf_w, wi_b, wo_b, wu_b, wf_b):
    f8 = np.float64
    x = np.asarray(x, np.float32)
    Wg = {"i": np.asarray(wi_w), "o": np.asarray(wo_w), "u": np.asarray(wu_w)}
    Bg = {"i": np.asarray(wi_b), "o": np.asarray(wo_b), "u": np.asarray(wu_b)}
    wf = np.asarray(wf_w)
    bf = np.asarray(wf_b)

    def t_tiles(a2d):
        # [512, n] -> [128p, 4t, n] with row r = t*128 + p
        return np.ascontiguousarray(a2d.reshape(4, 128, a2d.shape[1]).transpose(1, 0, 2))

    common = {}
    for g in "iuo":
        wx = np.ascontiguousarray(Wg[g][:, :IN].T).astype(np.float32)  # [512in, 512hid]
        if FP8_GATES[g]:
            wh_ = _q8(wx * SW)
            wl_ = _q8(wx * SW - wh_.astype(np.float32))
            pair = np.stack([wh_, wl_], axis=1)  # [512k, 2(hi,lo), 512m]
            common["w8" + g] = np.ascontiguousarray(
                pair.reshape(4, 128, 2, 512).transpose(1, 0, 2, 3)
            )
        else:
            common["wr" + g] = t_tiles(wx)
        common["wh" + g] = t_tiles(np.ascontiguousarray(Wg[g][:, IN:].T).astype(np.float32))
        common["bT" + g] = np.ascontiguousarray(np.asarray(Bg[g]).reshape(4, 128).T)
    common["wfh"] = t_tiles(np.ascontiguousarray(wf[:, IN:].T).astype(np.float32))

    # exact (f64) host precompute: level-3 x-part pre-activations and fx3
    X3 = np.asarray(x[OFFS[3] : OFFS[4]], f8)  # [4096, 512]
    fx3 = (X3 @ np.asarray(wf[:, :IN], f8).T + np.asarray(bf, f8)).astype(np.float32)
    l3pre = {
        g: (X3 @ np.asarray(Wg[g][:, :IN], f8).T + np.asarray(Bg[g], f8)).astype(np.float32)
        for g in "iuo"
    }

    in_maps = []
    for core in range(NC_N):
        m = dict(common)
        xl = x[OFFS[4] + LEAF_PER_CORE * core : OFFS[4] + LEAF_PER_CORE * (core + 1)]
        xlT = np.ascontiguousarray(xl.T)  # [512, 8192]
        tiles = xlT.reshape(4, 128, N_CHUNK, 512)  # [k, p, c, n]
        if any(FP8_GATES.values()):
            xh_ = _q8(tiles * SX)
            xlo = _q8(tiles * SX - xh_.astype(np.float32))
            # pair order (lo, hi); layout [128p, 16c, 4k, 2, 512n]
            m["x8"] = np.ascontiguousarray(
                np.stack([xlo, xh_], axis=3).transpose(1, 2, 0, 3, 4)
            )
        if not all(FP8_GATES.values()):
            m["xr"] = np.ascontiguousarray(tiles.transpose(1, 2, 0, 3))
        sl3 = slice(L3_PER_CORE * core, L3_PER_CORE * (core + 1))
        m["fx3b"] = t_tiles(np.ascontiguousarray(fx3[sl3].T))
        for g in "iuo":
            m["l3p" + g] = t_tiles(np.ascontiguousarray(l3pre[g][sl3].T))
        in_maps.append(m)
    return in_maps


def _t_to_nodes(a):
    """[128, 4, n] transposed tile -> [n, 512] natural (hid = t*128 + p)."""
    a = np.asarray(a)
    return np.ascontiguousarray(np.transpose(a, (2, 1, 0)).reshape(a.shape[2], 512))


def _host_finish(x, res, wi_w, wi_b, wf_w, wf_b, wo_w, wo_b, wu_w, wu_b):
    f8 = np.float64
    i3 = np.concatenate([_t_to_nodes(res[c]["i3T"]) for c in range(NC_N)]).astype(f8)
    u3 = np.concatenate([_t_to_nodes(res[c]["u3T"]) for c in range(NC_N)]).astype(f8)
    o3 = np.concatenate([_t_to_nodes(res[c]["o3T"]) for c in range(NC_N)]).astype(f8)
    fc3 = np.concatenate([_t_to_nodes(res[c]["fc3T"]) for c in range(NC_N)]).astype(f8)

    C = i3 * u3 + fc3
    H = o3 * np.tanh(C)

    wi = np.asarray(wi_w, f8)
    wo = np.asarray(wo_w, f8)
    wu = np.asarray(wu_w, f8)
    wf = np.asarray(wf_w, f8)
    bi, bo, bu, bf = (np.asarray(b, f8) for b in (wi_b, wo_b, wu_b, wf_b))

    for d in range(2, -1, -1):
        Xd = np.asarray(x[OFFS[d] : OFFS[d + 1]], f8)
        n = Xd.shape[0]
        Hc = H.reshape(n, BRANCH, HID)
        Cc = C.reshape(n, BRANCH, HID)
        h_sum = Hc.sum(axis=1)
        xh = np.concatenate([Xd, h_sum], axis=1)
        i = _np_sigmoid(xh @ wi.T + bi)
        o = _np_sigmoid(xh @ wo.T + bo)
        u = np.tanh(xh @ wu.T + bu)
        fx = Xd @ wf[:, :IN].T
        fc = (C @ wf[:, IN:].T).reshape(n, BRANCH, HID)
        f = _np_sigmoid(fc + fx[:, None, :] + bf)
        C = i * u + (f * Cc).sum(axis=1)
        H = o * np.tanh(C)

    return H[0].astype(np.float32), C[0].astype(np.float32)


def _run(in_maps, trace=False):
    from concourse.bass_utils import run_bass_kernel_spmd

    if "nc" not in _CACHE:
        _CACHE["nc"] = _build_nc()
    return run_bass_kernel_spmd(_CACHE["nc"], in_maps, list(range(NC_N)), trace=trace)


def kernel(x, wi_w, wi_b, wf_w, wf_b, wo_w, wo_b, wu_w, wu_b, _trace=False):
    x = np.asarray(x, np.float32)
    in_maps = _host_prep(x, wi_w, wo_w, wu_w, wf_w, wi_b, wo_b, wu_b, wf_b)
    res = _run(in_maps, trace=_trace)
    _CACHE["last_results"] = res
    H0, C0 = _host_finish(x, res.results, wi_w, wi_b, wf_w, wf_b, wo_w, wo_b, wu_w, wu_b)
    return H0, C0
